# revision 50
# baseline (speedup 1.0000x reference)
"""Trainium2 Bass kernel for nn_AdjacencyGenerator (gnn_message_passing).

Math note (verified against the reference to ~5e-7 rel err):
  The reference builds att = softmax(..., axis=1) over an [E, E, D] tensor and
  then contracts it with einsum('ijk,il->ikl', att, Wh).  Since the j index
  appears only in att and softmax normalizes over j, sum_j att[i,j,k] == 1
  exactly, so h_prime[i,k,l] == Wh[i,l].  Every op after that point is
  row-wise over the [E*D, D] view, and row i*D+k of that view is Wh[i,:]
  independent of k.  The whole attention tensor therefore cancels and the
  output is a per-edge scalar o[i] = f(Wh[i,:]) repeated D times.

  f is: elu -> LN(na) -> ff linear -> leaky -> LN(nf) -> wl linear -> leaky
        -> w5 linear -> +residual -> LN(fn) -> wv linear.

  Exact algebraic folds used on the host (none are approximations):
    * na_g/na_b fold into ff_w/ff_b            (LN -> Linear)
    * fn_g/fn_b fold into wv_w/wv_b            (LN -> Linear)
    * wl_b and w5_b fold jointly into the leaky shift bb and the t4 bias B,
      solving (I + wl_w @ w5_w) bb = wl_b - wl_w @ w5_b on the host — this
      removes all wl/w5 bias matmuls exactly.
    * elu is computed as elu(x)+1 = exp(min(x,0)) + max(x,0); the +1 shift
      is constant along the normalized axis so the following LN cancels it.

  rstd(var) = exp(-0.5*ln(var+eps)) on the scalar engine: ln and exp live in
  the same ACT table set, so the whole kernel uses exactly one table load,
  pre-warmed off the critical path.

Distribution: shard the E=1024 edges 128 per core across 8 NeuronCores,
data-parallel; all weights replicated.  The edge gather x[edge_index[1]] is
part of input sharding, done on the host.  Inputs ship as three packed
images: [xjT|W] (per-core), [ident|ffb], and one [128, 1284] weight image.
"""

import numpy as np

D = 128
E = 1024
NCORES = 8
PER = E // NCORES  # 128 edges per core
EPS = 1e-5

# column offsets inside the packed images
XW_XJT, XW_W = 0, 128                      # d_xw [128, 256] (per-core)
A_ID, A_FFB = 0, 128                       # d_wA [128, 256]
B_FFWT, B_WLWT, B_W5, B_WVR, B_NFG, B_NFB, B_BB3, B_WVB = (
    0, 128, 512, 896, 1024, 1152, 1280, 1283)
B_COLS = 1284

_CACHE = {}


class _Seq:
    """Sequential instruction emitter for one engine with semaphore tags.

    attach=True (single-instruction ops, DVE/ACT): one wait rides on the
    instruction's own sync_info (HW allows a single attached wait); any
    extra waits are emitted standalone.  attach=False (multi-instruction
    groups like matmul, and DMA): all waits are standalone so they gate the
    whole group.
    """

    def __init__(self, eng, sem, all_self_waits, attach=False):
        self.eng, self.sem, self.n = eng, sem, 0
        self.all_self_waits = all_self_waits
        self.attach = attach

    def emit(self, make, waits=(), self_wait=False):
        allw = list(waits)
        if (self_wait or self.all_self_waits) and self.n:
            allw.append((self.sem, self.n))
        if self.attach and allw:
            for s, v in allw[:-1]:
                self.eng.wait_ge(s, v)
            inst = make()
            inst._wait_ge(*allw[-1])
        else:
            for s, v in allw:
                self.eng.wait_ge(s, v)
            inst = make()
        inst.then_inc(self.sem, 1)
        self.n += 1
        return self.n


def _build_nc(validation=False):
    import concourse.bass as bass
    from concourse import mybir

    f32 = mybir.dt.float32
    Alu = mybir.AluOpType
    Act = mybir.ActivationFunctionType

    nc = bass.Bass(detect_race_conditions=validation)

    d_xw = nc.dram_tensor("xw", [128, 256], f32, kind="ExternalInput")
    d_wA = nc.dram_tensor("wpacka", [128, 256], f32, kind="ExternalInput")
    d_wB = nc.dram_tensor("wpackb", [128, B_COLS], f32, kind="ExternalInput")
    d_out = nc.dram_tensor("out", [PER, 1], f32, kind="ExternalOutput")

    from contextlib import ExitStack

    ctx = ExitStack()
    sb = lambda name, shape, dt=f32: ctx.enter_context(
        nc.sbuf_tensor(name, shape, dt))
    ps = lambda name, shape: ctx.enter_context(nc.psum_tensor(name, shape, f32))

    s_xj = sb("s_xj", [128, 128])
    s_w = sb("s_w", [128, 128])
    s_wA = sb("s_wa", [128, 256])
    s_wB = sb("s_wb", [128, B_COLS])

    ones = sb("ones", [1, 128])
    epsc = sb("epsc", [PER, 1])
    zeroc = sb("zeroc", [PER, 1])
    m0 = sb("m0", [PER, D])        # min(Wh, 0)
    ex = sb("ex", [PER, D])        # exp(min(Wh, 0))
    t1 = sb("t1", [PER, D])        # elu(Wh) + 1
    t2 = sb("t2", [PER, D])        # LN1 core
    t2T = sb("t2t", [D, PER])
    lk1 = sb("lk1", [PER, D])
    t3 = sb("t3", [PER, D])        # leaky(ff out)
    u = sb("u", [PER, D])          # LN2 core
    t4a = sb("t4a", [PER, D])
    t4 = sb("t4", [PER, D])
    t4T = sb("t4t", [D, PER])
    lka = sb("lka", [128, 3, PER])
    y1T = sb("y1t", [128, 3, PER])
    y3 = sb("y3", [PER, D])
    y4 = sb("y4", [PER, D])        # LN3 core
    y4w = sb("y4w", [PER, D])
    ocol = sb("ocol", [PER, 1])
    st = sb("st", [PER, 6])        # LN scratch (reused by all three LNs)
    mv = sb("mv", [PER, 2])
    lnv = sb("lnv", [PER, 1])
    rstd = sb("rstd", [PER, 1])
    scr = sb("scr", [1, 1])        # ACT warmup scratch

    p_wh = ps("p_wh", [PER, D])
    p_t2T = ps("p_t2t", [D, PER])
    p_q2 = ps("p_q2", [PER, D])
    p_t4T = ps("p_t4t", [D, PER])
    p_y1T = [ps(f"p_y1t{c}", [128, PER]) for c in range(3)]
    p_y2 = ps("p_y2", [PER, D])

    dsem_x = ctx.enter_context(nc.semaphore("dsem_x"))
    dsem_y = ctx.enter_context(nc.semaphore("dsem_y"))
    dsem_o = ctx.enter_context(nc.semaphore("dsem_o"))
    dsem_a = ctx.enter_context(nc.semaphore("dsem_a"))
    dsem_b = ctx.enter_context(nc.semaphore("dsem_b"))
    psem = ctx.enter_context(nc.semaphore("psem"))
    vsem = ctx.enter_context(nc.semaphore("vsem"))
    asem = ctx.enter_context(nc.semaphore("asem"))
    gsem = ctx.enter_context(nc.semaphore("gsem"))

    # ---- vector op indices ----------------------------------------------
    V_M0, V_T1 = 1, 2
    V_ST1, V_MV1, V_T2 = 3, 4, 5
    V_T2T, V_LK1, V_T3 = 6, 7, 8
    V_ST2, V_MV2, V_U = 9, 10, 11
    V_T4A, V_T4, V_T4T = 12, 13, 14
    V_Y1T = [16, 18, 20]
    V_Y3 = 21
    V_ST3, V_MV3, V_Y4 = 22, 23, 24
    V_Y4W, V_OCOL, V_OSB = 25, 26, 26
    # ---- PE op indices ---------------------------------------------------
    P_WH, P_Q2B, P_T2T, P_Q2, P_T4T = 1, 2, 3, 4, 5
    P_WL = [6, 7, 8]
    P_Y2 = [9, 10, 11]
    # ---- ACT op indices --------------------------------------------------
    A_WARM, A_EX = 1, 2
    A_R1, A_R2, A_R3 = 4, 6, 8
    # ---- gpsimd ----------------------------------------------------------
    G_ONES, G_SETUP = 1, 3

    with nc.Block() as block:

        @block.sync
        def _(sync):
            sync.dma_start(out=s_w[:, :], in_=d_xw[:, XW_W:XW_W + 128]
                           ).then_inc(dsem_y, 16)
            sync.dma_start(out=s_wA[:, :], in_=d_wA[:, :]).then_inc(dsem_a, 16)
            sync.dma_start(out=s_wB[:, :], in_=d_wB[:, :]).then_inc(dsem_b, 16)
            sync.wait_ge(vsem, V_OSB)
            sync.dma_start(out=d_out[:, :], in_=ocol[:, :]).then_inc(dsem_o, 16)
            sync.wait_ge(dsem_o, 16)

        @block.gpsimd
        def _(ge):
            ge.dma_start(out=s_xj[:, :], in_=d_xw[:, XW_XJT:XW_XJT + 128]
                         ).then_inc(dsem_x, 16)
            ge.memset(ones[:, :], 1.0).then_inc(gsem, 1)
            ge.memset(epsc[:, :], EPS).then_inc(gsem, 1)
            ge.memset(zeroc[:, :], 0.0).then_inc(gsem, 1)

        @block.scalar
        def _(se):
            A = _Seq(se, asem, validation, attach=True)
            # pre-warm the ln/exp table set off the critical path
            A.emit(lambda: se.activation(out=scr[:, :], in_=ones[0:1, 0:1],
                                         func=Act.Ln),
                   waits=[(gsem, G_ONES)])
            A.emit(lambda: se.activation(out=ex[:, :], in_=m0[:, :],
                                         func=Act.Exp),
                   waits=[(vsem, V_M0)])
            assert A.n == A_EX
            for a_idx, v_mv in ((A_R1, V_MV1), (A_R2, V_MV2), (A_R3, V_MV3)):
                # rstd = exp(-0.5 * ln(var + eps))
                A.emit(lambda v_mv=v_mv: se.activation(
                    out=lnv[:, :], in_=mv[:, 1:2], func=Act.Ln,
                    bias=epsc[:, 0:1]),
                    waits=[(vsem, v_mv)])
                A.emit(lambda: se.activation(out=rstd[:, :], in_=lnv[:, :],
                                             func=Act.Exp, scale=-0.5),
                       self_wait=True)
                assert A.n == a_idx

        @block.tensor
        def _(te):
            T = _Seq(te, psem, validation)
            # Wh = xj @ W  (xjT and W arrive on different DMA rings)
            T.emit(lambda: te.matmul(p_wh[:, :], s_xj[:, :], s_w[:, :],
                                     start=True, stop=True),
                   waits=[(dsem_x, 16), (dsem_y, 16)])
            # ff bias early (its only deps are DMA + ones memset)
            T.emit(lambda: te.matmul(p_q2[:, :], ones[:, :],
                                     s_wA[0:1, A_FFB:A_FFB + 128],
                                     start=True, stop=False,
                                     skip_group_check=True),
                   waits=[(dsem_a, 16), (dsem_b, 16), (gsem, G_ONES)])
            T.emit(lambda: te.transpose(p_t2T[:, :], t2[:, :],
                                        s_wA[:, A_ID:A_ID + 128]),
                   waits=[(vsem, V_T2)])
            T.emit(lambda: te.matmul(p_q2[:, :], t2T[:, :],
                                     s_wB[:, B_FFWT:B_FFWT + 128],
                                     start=False, stop=True,
                                     skip_group_check=True),
                   waits=[(vsem, V_T2T)])
            T.emit(lambda: te.transpose(p_t4T[:, :], t4[:, :],
                                        s_wA[:, A_ID:A_ID + 128]),
                   waits=[(vsem, V_T4)])
            for c in range(3):
                T.emit(lambda c=c: te.matmul(
                    p_y1T[c][:, :],
                    s_wB[:, B_WLWT + c * 128:B_WLWT + (c + 1) * 128],
                    t4T[:, :], start=True, stop=True),
                    waits=[(vsem, V_T4T)] if c == 0 else ())
            for c in range(3):
                T.emit(lambda c=c: te.matmul(
                    p_y2[:, :], y1T[:, c, :],
                    s_wB[:, B_W5 + c * 128:B_W5 + (c + 1) * 128],
                    start=(c == 0), stop=(c == 2)),
                    waits=[(vsem, V_Y1T[c])])
            assert T.n == P_Y2[2]

        @block.vector
        def _(ve):
            V = _Seq(ve, vsem, validation, attach=True)
            V.emit(lambda: ve.tensor_scalar_min(out=m0[:, :], in0=p_wh[:, :],
                                                scalar1=0.0),
                   waits=[(psem, P_WH), (gsem, G_SETUP)])
            V.emit(lambda: ve.scalar_tensor_tensor(out=t1[:, :], in0=p_wh[:, :],
                                                   scalar=0.0, in1=ex[:, :],
                                                   op0=Alu.max, op1=Alu.add),
                   waits=[(asem, A_EX)])
            assert V.n == V_T1

            def ln_core(src, dst, a_idx, v_stats):
                V.emit(lambda: ve.bn_stats(out=st[:, :], in_=src[:, :]))
                V.emit(lambda: ve.bn_aggr(out=mv[:, :], in_=st[:, :]),
                       self_wait=True)
                assert V.n == v_stats + 1
                # scalar operands latch at dispatch; the asem wait (ACT wrote
                # rstd) transitively guarantees mv is long since drained
                V.emit(lambda: ve.tensor_scalar(out=dst[:, :], in0=src[:, :],
                                                scalar1=mv[:, 0:1],
                                                scalar2=rstd[:, 0:1],
                                                op0=Alu.subtract,
                                                op1=Alu.mult),
                       waits=[(asem, a_idx)])

            ln_core(t1, t2, A_R1, V_ST1)
            assert V.n == V_T2
            V.emit(lambda: ve.tensor_copy(out=t2T[:, :], in_=p_t2T[:, :]),
                   waits=[(psem, P_T2T)])
            # leaky(q2) = q2 - 0.8*min(q2, 0)
            V.emit(lambda: ve.tensor_scalar(out=lk1[:, :], in0=p_q2[:, :],
                                            scalar1=0.0, scalar2=0.8,
                                            op0=Alu.min, op1=Alu.mult),
                   waits=[(psem, P_Q2)])
            V.emit(lambda: ve.tensor_sub(out=t3[:, :], in0=p_q2[:, :],
                                         in1=lk1[:, :]))
            assert V.n == V_T3
            ln_core(t3, u, A_R2, V_ST2)
            assert V.n == V_U
            # t4 = u * nf_g + B
            V.emit(lambda: ve.tensor_mul(out=t4a[:, :], in0=u[:, :],
                                         in1=s_wB[:, B_NFG:B_NFG + 128]))
            V.emit(lambda: ve.tensor_add(out=t4[:, :], in0=t4a[:, :],
                                         in1=s_wB[:, B_NFB:B_NFB + 128]))
            V.emit(lambda: ve.tensor_copy(out=t4T[:, :], in_=p_t4T[:, :]),
                   waits=[(psem, P_T4T)])
            assert V.n == V_T4T
            # leaky with folded bias, per chunk (each wl output has its
            # own PSUM bank, so chunk c can be read while the PE writes c+1):
            #   y1T_c = mm_c - 0.8*min(mm_c + bb_c, 0)
            for c in range(3):
                bb_c = s_wB[:, B_BB3 + c:B_BB3 + c + 1]
                V.emit(lambda c=c, bb_c=bb_c: ve.tensor_scalar(
                    out=lka[:, c, :], in0=p_y1T[c][:, :],
                    scalar1=bb_c, scalar2=zeroc[:, 0:1],
                    op0=Alu.add, op1=Alu.min),
                    waits=[(psem, P_WL[c])])
                V.emit(lambda c=c: ve.scalar_tensor_tensor(
                    out=y1T[:, c, :], in0=lka[:, c, :], scalar=-0.8,
                    in1=p_y1T[c][:, :], op0=Alu.mult, op1=Alu.add))
                assert V.n == V_Y1T[c]
            V.emit(lambda: ve.tensor_add(out=y3[:, :], in0=p_y2[:, :],
                                         in1=t4[:, :]),
                   waits=[(psem, P_Y2[2])])
            assert V.n == V_Y3
            # LN3 fused with the wv dot product:
            #   out[e] = rstd3[e] * sum_k ((y3-m3)[e,k]*wv_eff[k]) + wvb
            # the (y3-m)*wv part runs on DVE while ACT computes rstd3
            V.emit(lambda: ve.bn_stats(out=st[:, :], in_=y3[:, :]))
            V.emit(lambda: ve.bn_aggr(out=mv[:, :], in_=st[:, :]),
                   self_wait=True)
            assert V.n == V_MV3
            V.emit(lambda: ve.scalar_tensor_tensor(
                out=y4w[:, :], in0=y3[:, :], scalar=mv[:, 0:1],
                in1=s_wB[:, B_WVR:B_WVR + 128],
                op0=Alu.subtract, op1=Alu.mult),
                self_wait=True)
            V.emit(lambda: ve.tensor_reduce(out=ocol[:, :], in_=y4w[:, :],
                                            axis=mybir.AxisListType.X,
                                            op=Alu.add))
            V.emit(lambda: ve.tensor_scalar(out=ocol[:, :], in0=ocol[:, :],
                                            scalar1=rstd[:, 0:1],
                                            scalar2=s_wB[:, B_WVB:B_WVB + 1],
                                            op0=Alu.mult, op1=Alu.add),
                   waits=[(asem, A_R3)])
            assert V.n == V_OSB

    return nc, ctx


def _get_nc(validation=False):
    key = "ncv" if validation else "nc"
    if key not in _CACHE:
        _CACHE[key] = _build_nc(validation)
    return _CACHE[key][0]


def _prep_in_maps(inputs):
    """Host-side sharding + exact algebraic weight folding + packing."""
    g = lambda k: np.asarray(inputs[k], dtype=np.float64)
    x = g("x")
    ei = np.asarray(inputs["edge_index"]).astype(np.int64)
    W = g("W")
    ff_w, ff_b = g("ff_w"), g("ff_b")
    na_g, na_b = g("na_g"), g("na_b")
    nf_g, nf_b = g("nf_g"), g("nf_b")
    wl_w, wl_b = g("wl_w"), g("wl_b")
    w5_w, w5_b = g("w5_w"), g("w5_b")
    fn_g, fn_b = g("fn_g"), g("fn_b")
    wv_w, wv_b = g("wv_w"), g("wv_b")

    xj = x[ei[1]]                           # [E, D] gather on host
    ffw_eff = ff_w * na_g[None, :]          # fold LN(na) scale into ff
    ffb_eff = ff_b + ff_w @ na_b            # fold LN(na) bias into ff
    wv_eff = wv_w[0] * fn_g                 # fold LN(fn) scale into wv
    wvb_eff = wv_b[0] + wv_w[0] @ fn_b      # fold LN(fn) bias into wv
    # joint fold of wl_b and w5_b into the leaky shift bb and t4 bias B:
    #   bb = wl_b - wl_w @ (B - nf_b),  B - nf_b = w5_b + w5_w @ bb
    bb = np.linalg.solve(np.eye(3 * D) + wl_w @ w5_w, wl_b - wl_w @ w5_b)
    B_bias = nf_b + w5_b + w5_w @ bb

    wA = np.zeros((128, 256), np.float64)
    wA[:, A_ID:A_ID + 128] = np.eye(128)
    wA[0, A_FFB:A_FFB + 128] = ffb_eff

    wB = np.zeros((128, B_COLS), np.float64)
    wB[:, B_FFWT:B_FFWT + 128] = ffw_eff.T
    wB[:, B_WLWT:B_WLWT + 384] = wl_w.T
    wB[:, B_W5:B_W5 + 384] = w5_w.T.reshape(3, 128, 128).transpose(
        1, 0, 2).reshape(128, 384)
    wB[:, B_WVR:B_WVR + 128] = wv_eff[None, :]
    wB[:, B_NFG:B_NFG + 128] = nf_g[None, :]
    wB[:, B_NFB:B_NFB + 128] = B_bias[None, :]
    wB[:, B_BB3:B_BB3 + 3] = bb.reshape(3, 128).T
    wB[:, B_WVB] = wvb_eff

    f32 = lambda a: np.ascontiguousarray(a, dtype=np.float32)
    shared = {"wpacka": f32(wA), "wpackb": f32(wB)}
    in_maps = []
    for c in range(NCORES):
        xw = np.empty((128, 256), np.float64)
        xw[:, XW_XJT:XW_XJT + 128] = xj[c * PER:(c + 1) * PER].T
        xw[:, XW_W:XW_W + 128] = W
        m = dict(shared)
        m["xw"] = f32(xw)
        in_maps.append(m)
    return in_maps


def kernel(**inputs) -> np.ndarray:
    from concourse.bass_utils import run_bass_kernel_spmd

    nc = _get_nc()
    in_maps = _prep_in_maps(inputs)
    res = run_bass_kernel_spmd(nc, in_maps, core_ids=list(range(NCORES)))
    o = np.concatenate(
        [np.asarray(res.results[c]["out"]).reshape(-1) for c in range(NCORES)]
    )
    return np.repeat(o, D)  # each per-edge scalar spans D output slots


# revision 51
# speedup vs baseline: 1.1499x; 1.1499x over previous
"""Trainium2 Bass kernel for nn_AdjacencyGenerator (gnn_message_passing).

Math note (verified against the reference to ~5e-7 rel err):
  The reference builds att = softmax(..., axis=1) over an [E, E, D] tensor and
  then contracts it with einsum('ijk,il->ikl', att, Wh).  Since the j index
  appears only in att and softmax normalizes over j, sum_j att[i,j,k] == 1
  exactly, so h_prime[i,k,l] == Wh[i,l].  Every op after that point is
  row-wise over the [E*D, D] view, and row i*D+k of that view is Wh[i,:]
  independent of k.  The whole attention tensor therefore cancels and the
  output is a per-edge scalar o[i] = f(Wh[i,:]) repeated D times.

  f is: elu -> LN(na) -> ff linear -> leaky -> LN(nf) -> wl linear -> leaky
        -> w5 linear -> +residual -> LN(fn) -> wv linear.

  Exact algebraic folds used on the host (none are approximations):
    * na_g/na_b fold into ff_w/ff_b            (LN -> Linear)
    * fn_g/fn_b fold into wv_w/wv_b            (LN -> Linear)
    * wl_b and w5_b fold jointly into the leaky shift bb and the t4 bias B,
      solving (I + wl_w @ w5_w) bb = wl_b - wl_w @ w5_b on the host — this
      removes all wl/w5 bias matmuls exactly.
    * elu is computed as elu(x)+1 = exp(min(x,0)) + max(x,0); the +1 shift
      is constant along the normalized axis so the following LN cancels it.

  rstd(var) = exp(-0.5*ln(var+eps)) on the scalar engine: ln and exp live in
  the same ACT table set, so the whole kernel uses exactly one table load,
  pre-warmed off the critical path.

Distribution: shard the E=1024 edges 128 per core across 8 NeuronCores,
data-parallel; all weights replicated.  The edge gather x[edge_index[1]] is
part of input sharding, done on the host.  Inputs ship as three packed
images: [xjT|W] (per-core), [ident|ffb], and one [128, 1284] weight image.
"""

import numpy as np

D = 128
E = 1024
NCORES = 8
PER = E // NCORES  # 128 edges per core
EPS = 1e-5

# column offsets inside the packed images
XW_XJT, XW_W = 0, 128                      # d_xw [128, 256] (per-core)
A_ID, A_FFB = 0, 128                       # d_wA [128, 256]
B_FFWT, B_WLWT, B_W5, B_WVR, B_NFG, B_NFB, B_BB3, B_WVB = (
    0, 128, 512, 896, 1024, 1152, 1280, 1283)
B_COLS = 1284

_CACHE = {}


class _Seq:
    """Sequential instruction emitter for one engine with semaphore tags.

    attach=True (single-instruction ops, DVE/ACT): one wait rides on the
    instruction's own sync_info (HW allows a single attached wait); any
    extra waits are emitted standalone.  attach=False (multi-instruction
    groups like matmul, and DMA): all waits are standalone so they gate the
    whole group.
    """

    def __init__(self, eng, sem, all_self_waits, attach=False):
        self.eng, self.sem, self.n = eng, sem, 0
        self.all_self_waits = all_self_waits
        self.attach = attach

    def emit(self, make, waits=(), self_wait=False):
        allw = list(waits)
        if (self_wait or self.all_self_waits) and self.n:
            allw.append((self.sem, self.n))
        if self.attach and allw:
            for s, v in allw[:-1]:
                self.eng.wait_ge(s, v)
            inst = make()
            inst._wait_ge(*allw[-1])
        else:
            for s, v in allw:
                self.eng.wait_ge(s, v)
            inst = make()
        inst.then_inc(self.sem, 1)
        self.n += 1
        return self.n


def _build_nc(validation=False):
    import concourse.bass as bass
    from concourse import mybir

    f32 = mybir.dt.float32
    Alu = mybir.AluOpType
    Act = mybir.ActivationFunctionType

    nc = bass.Bass(detect_race_conditions=validation)

    d_xw = nc.dram_tensor("xw", [128, 256], f32, kind="ExternalInput")
    d_wA = nc.dram_tensor("wpacka", [128, 256], f32, kind="ExternalInput")
    d_wB = nc.dram_tensor("wpackb", [128, B_COLS], f32, kind="ExternalInput")
    d_out = nc.dram_tensor("out", [PER, D], f32, kind="ExternalOutput")

    from contextlib import ExitStack

    ctx = ExitStack()
    sb = lambda name, shape, dt=f32: ctx.enter_context(
        nc.sbuf_tensor(name, shape, dt))
    ps = lambda name, shape: ctx.enter_context(nc.psum_tensor(name, shape, f32))

    s_xj = sb("s_xj", [128, 128])
    s_w = sb("s_w", [128, 128])
    s_wA = sb("s_wa", [128, 256])
    s_wB = sb("s_wb", [128, B_COLS])

    ones = sb("ones", [1, 128])
    epsc = sb("epsc", [PER, 1])
    zeroc = sb("zeroc", [PER, 1])
    m0 = sb("m0", [PER, D])        # min(Wh, 0)
    ex = sb("ex", [PER, D])        # exp(min(Wh, 0))
    t1 = sb("t1", [PER, D])        # elu(Wh) + 1
    t2 = sb("t2", [PER, D])        # LN1 core
    t2T = sb("t2t", [D, PER])
    lk1 = sb("lk1", [PER, D])
    t3 = sb("t3", [PER, D])        # leaky(ff out)
    u = sb("u", [PER, D])          # LN2 core
    t4a = sb("t4a", [PER, D])
    t4 = sb("t4", [PER, D])
    t4T = sb("t4t", [D, PER])
    lka = sb("lka", [128, 3, PER])
    y1T = sb("y1t", [128, 3, PER])
    y3 = sb("y3", [PER, D])
    y4 = sb("y4", [PER, D])        # LN3 core
    y4w = sb("y4w", [PER, D])
    ocol = sb("ocol", [PER, 1])
    zerot = sb("zerot", [PER, D])
    o_sb = sb("o_sb", [PER, D])
    st = sb("st", [PER, 6])        # LN scratch (reused by all three LNs)
    mv = sb("mv", [PER, 2])
    lnv = sb("lnv", [PER, 1])
    rstd = sb("rstd", [PER, 1])
    scr = sb("scr", [1, 1])        # ACT warmup scratch

    p_wh = ps("p_wh", [PER, D])
    p_t2T = ps("p_t2t", [D, PER])
    p_q2 = ps("p_q2", [PER, D])
    p_t4T = ps("p_t4t", [D, PER])
    p_y1T = [ps(f"p_y1t{c}", [128, PER]) for c in range(3)]
    p_y2 = ps("p_y2", [PER, D])

    dsem_x = ctx.enter_context(nc.semaphore("dsem_x"))
    dsem_y = ctx.enter_context(nc.semaphore("dsem_y"))
    dsem_o = ctx.enter_context(nc.semaphore("dsem_o"))
    dsem_a = ctx.enter_context(nc.semaphore("dsem_a"))
    dsem_b = ctx.enter_context(nc.semaphore("dsem_b"))
    psem = ctx.enter_context(nc.semaphore("psem"))
    vsem = ctx.enter_context(nc.semaphore("vsem"))
    asem = ctx.enter_context(nc.semaphore("asem"))
    gsem = ctx.enter_context(nc.semaphore("gsem"))

    # ---- vector op indices ----------------------------------------------
    V_M0, V_T1 = 1, 2
    V_ST1, V_MV1, V_T2 = 3, 4, 5
    V_T2T, V_LK1, V_T3 = 6, 7, 8
    V_ST2, V_MV2, V_U = 9, 10, 11
    V_T4A, V_T4, V_T4T = 12, 13, 14
    V_Y1T = [16, 18, 20]
    V_Y3 = 21
    V_ST3, V_MV3, V_Y4 = 22, 23, 24
    V_Y4W, V_OCOL, V_OSB = 25, 26, 27
    # ---- PE op indices ---------------------------------------------------
    P_WH, P_Q2B, P_T2T, P_Q2, P_T4T = 1, 2, 3, 4, 5
    P_WL = [6, 7, 8]
    P_Y2 = [9, 10, 11]
    # ---- ACT op indices --------------------------------------------------
    A_WARM, A_EX = 1, 2
    A_R1, A_R2, A_R3 = 4, 6, 8
    # ---- gpsimd ----------------------------------------------------------
    G_ONES, G_SETUP = 1, 4

    with nc.Block() as block:

        @block.sync
        def _(sync):
            sync.dma_start(out=s_w[:, :], in_=d_xw[:, XW_W:XW_W + 128]
                           ).then_inc(dsem_y, 16)
            sync.dma_start(out=s_wA[:, :], in_=d_wA[:, :]).then_inc(dsem_a, 16)
            sync.dma_start(out=s_wB[:, :], in_=d_wB[:, :]).then_inc(dsem_b, 16)
            sync.wait_ge(vsem, V_OSB)
            sync.dma_start(out=d_out[:, :], in_=o_sb[:, :]).then_inc(dsem_o, 16)
            sync.wait_ge(dsem_o, 16)

        @block.gpsimd
        def _(ge):
            ge.dma_start(out=s_xj[:, :], in_=d_xw[:, XW_XJT:XW_XJT + 128]
                         ).then_inc(dsem_x, 16)
            ge.memset(ones[:, :], 1.0).then_inc(gsem, 1)
            ge.memset(epsc[:, :], EPS).then_inc(gsem, 1)
            ge.memset(zeroc[:, :], 0.0).then_inc(gsem, 1)
            ge.memset(zerot[:, :], 0.0).then_inc(gsem, 1)

        @block.scalar
        def _(se):
            A = _Seq(se, asem, validation, attach=True)
            # pre-warm the ln/exp table set off the critical path
            A.emit(lambda: se.activation(out=scr[:, :], in_=ones[0:1, 0:1],
                                         func=Act.Ln),
                   waits=[(gsem, G_ONES)])
            A.emit(lambda: se.activation(out=ex[:, :], in_=m0[:, :],
                                         func=Act.Exp),
                   waits=[(vsem, V_M0)])
            assert A.n == A_EX
            for a_idx, v_mv in ((A_R1, V_MV1), (A_R2, V_MV2), (A_R3, V_MV3)):
                # rstd = exp(-0.5 * ln(var + eps))
                A.emit(lambda v_mv=v_mv: se.activation(
                    out=lnv[:, :], in_=mv[:, 1:2], func=Act.Ln,
                    bias=epsc[:, 0:1]),
                    waits=[(vsem, v_mv)])
                A.emit(lambda: se.activation(out=rstd[:, :], in_=lnv[:, :],
                                             func=Act.Exp, scale=-0.5),
                       self_wait=True)
                assert A.n == a_idx

        @block.tensor
        def _(te):
            T = _Seq(te, psem, validation)
            # Wh = xj @ W  (xjT and W arrive on different DMA rings)
            T.emit(lambda: te.matmul(p_wh[:, :], s_xj[:, :], s_w[:, :],
                                     start=True, stop=True),
                   waits=[(dsem_x, 16), (dsem_y, 16)])
            # ff bias early (its only deps are DMA + ones memset)
            T.emit(lambda: te.matmul(p_q2[:, :], ones[:, :],
                                     s_wA[0:1, A_FFB:A_FFB + 128],
                                     start=True, stop=False,
                                     skip_group_check=True),
                   waits=[(dsem_a, 16), (dsem_b, 16), (gsem, G_ONES)])
            T.emit(lambda: te.transpose(p_t2T[:, :], t2[:, :],
                                        s_wA[:, A_ID:A_ID + 128]),
                   waits=[(vsem, V_T2)])
            T.emit(lambda: te.matmul(p_q2[:, :], t2T[:, :],
                                     s_wB[:, B_FFWT:B_FFWT + 128],
                                     start=False, stop=True,
                                     skip_group_check=True),
                   waits=[(vsem, V_T2T)])
            T.emit(lambda: te.transpose(p_t4T[:, :], t4[:, :],
                                        s_wA[:, A_ID:A_ID + 128]),
                   waits=[(vsem, V_T4)])
            for c in range(3):
                T.emit(lambda c=c: te.matmul(
                    p_y1T[c][:, :],
                    s_wB[:, B_WLWT + c * 128:B_WLWT + (c + 1) * 128],
                    t4T[:, :], start=True, stop=True),
                    waits=[(vsem, V_T4T)] if c == 0 else ())
            for c in range(3):
                T.emit(lambda c=c: te.matmul(
                    p_y2[:, :], y1T[:, c, :],
                    s_wB[:, B_W5 + c * 128:B_W5 + (c + 1) * 128],
                    start=(c == 0), stop=(c == 2)),
                    waits=[(vsem, V_Y1T[c])])
            assert T.n == P_Y2[2]

        @block.vector
        def _(ve):
            V = _Seq(ve, vsem, validation, attach=True)
            V.emit(lambda: ve.tensor_scalar_min(out=m0[:, :], in0=p_wh[:, :],
                                                scalar1=0.0),
                   waits=[(psem, P_WH), (gsem, G_SETUP)])
            V.emit(lambda: ve.scalar_tensor_tensor(out=t1[:, :], in0=p_wh[:, :],
                                                   scalar=0.0, in1=ex[:, :],
                                                   op0=Alu.max, op1=Alu.add),
                   waits=[(asem, A_EX)])
            assert V.n == V_T1

            def ln_core(src, dst, a_idx, v_stats):
                V.emit(lambda: ve.bn_stats(out=st[:, :], in_=src[:, :]))
                V.emit(lambda: ve.bn_aggr(out=mv[:, :], in_=st[:, :]),
                       self_wait=True)
                assert V.n == v_stats + 1
                # scalar operands latch at dispatch; the asem wait (ACT wrote
                # rstd) transitively guarantees mv is long since drained
                V.emit(lambda: ve.tensor_scalar(out=dst[:, :], in0=src[:, :],
                                                scalar1=mv[:, 0:1],
                                                scalar2=rstd[:, 0:1],
                                                op0=Alu.subtract,
                                                op1=Alu.mult),
                       waits=[(asem, a_idx)])

            ln_core(t1, t2, A_R1, V_ST1)
            assert V.n == V_T2
            V.emit(lambda: ve.tensor_copy(out=t2T[:, :], in_=p_t2T[:, :]),
                   waits=[(psem, P_T2T)])
            # leaky(q2) = q2 - 0.8*min(q2, 0)
            V.emit(lambda: ve.tensor_scalar(out=lk1[:, :], in0=p_q2[:, :],
                                            scalar1=0.0, scalar2=0.8,
                                            op0=Alu.min, op1=Alu.mult),
                   waits=[(psem, P_Q2)])
            V.emit(lambda: ve.tensor_sub(out=t3[:, :], in0=p_q2[:, :],
                                         in1=lk1[:, :]))
            assert V.n == V_T3
            ln_core(t3, u, A_R2, V_ST2)
            assert V.n == V_U
            # t4 = u * nf_g + B
            V.emit(lambda: ve.tensor_mul(out=t4a[:, :], in0=u[:, :],
                                         in1=s_wB[:, B_NFG:B_NFG + 128]))
            V.emit(lambda: ve.tensor_add(out=t4[:, :], in0=t4a[:, :],
                                         in1=s_wB[:, B_NFB:B_NFB + 128]))
            V.emit(lambda: ve.tensor_copy(out=t4T[:, :], in_=p_t4T[:, :]),
                   waits=[(psem, P_T4T)])
            assert V.n == V_T4T
            # leaky with folded bias, per chunk (each wl output has its
            # own PSUM bank, so chunk c can be read while the PE writes c+1):
            #   y1T_c = mm_c - 0.8*min(mm_c + bb_c, 0)
            for c in range(3):
                bb_c = s_wB[:, B_BB3 + c:B_BB3 + c + 1]
                V.emit(lambda c=c, bb_c=bb_c: ve.tensor_scalar(
                    out=lka[:, c, :], in0=p_y1T[c][:, :],
                    scalar1=bb_c, scalar2=zeroc[:, 0:1],
                    op0=Alu.add, op1=Alu.min),
                    waits=[(psem, P_WL[c])])
                V.emit(lambda c=c: ve.scalar_tensor_tensor(
                    out=y1T[:, c, :], in0=lka[:, c, :], scalar=-0.8,
                    in1=p_y1T[c][:, :], op0=Alu.mult, op1=Alu.add))
                assert V.n == V_Y1T[c]
            V.emit(lambda: ve.tensor_add(out=y3[:, :], in0=p_y2[:, :],
                                         in1=t4[:, :]),
                   waits=[(psem, P_Y2[2])])
            assert V.n == V_Y3
            # LN3 fused with the wv dot product:
            #   out[e] = rstd3[e] * sum_k ((y3-m3)[e,k]*wv_eff[k]) + wvb
            # the (y3-m)*wv part runs on DVE while ACT computes rstd3
            V.emit(lambda: ve.bn_stats(out=st[:, :], in_=y3[:, :]))
            V.emit(lambda: ve.bn_aggr(out=mv[:, :], in_=st[:, :]),
                   self_wait=True)
            assert V.n == V_MV3
            V.emit(lambda: ve.scalar_tensor_tensor(
                out=y4w[:, :], in0=y3[:, :], scalar=mv[:, 0:1],
                in1=s_wB[:, B_WVR:B_WVR + 128],
                op0=Alu.subtract, op1=Alu.mult),
                self_wait=True)
            V.emit(lambda: ve.tensor_reduce(out=ocol[:, :], in_=y4w[:, :],
                                            axis=mybir.AxisListType.X,
                                            op=Alu.add))
            V.emit(lambda: ve.tensor_scalar(out=ocol[:, :], in0=ocol[:, :],
                                            scalar1=rstd[:, 0:1],
                                            scalar2=s_wB[:, B_WVB:B_WVB + 1],
                                            op0=Alu.mult, op1=Alu.add),
                   waits=[(asem, A_R3)])
            V.emit(lambda: ve.tensor_scalar_add(out=o_sb[:, :],
                                                in0=zerot[:, :],
                                                scalar1=ocol[:, 0:1]),
                   self_wait=True)
            assert V.n == V_OSB

    return nc, ctx


def _get_nc(validation=False):
    key = "ncv" if validation else "nc"
    if key not in _CACHE:
        _CACHE[key] = _build_nc(validation)
    return _CACHE[key][0]


def _prep_in_maps(inputs):
    """Host-side sharding + exact algebraic weight folding + packing."""
    g = lambda k: np.asarray(inputs[k], dtype=np.float64)
    x = g("x")
    ei = np.asarray(inputs["edge_index"]).astype(np.int64)
    W = g("W")
    ff_w, ff_b = g("ff_w"), g("ff_b")
    na_g, na_b = g("na_g"), g("na_b")
    nf_g, nf_b = g("nf_g"), g("nf_b")
    wl_w, wl_b = g("wl_w"), g("wl_b")
    w5_w, w5_b = g("w5_w"), g("w5_b")
    fn_g, fn_b = g("fn_g"), g("fn_b")
    wv_w, wv_b = g("wv_w"), g("wv_b")

    xj = x[ei[1]]                           # [E, D] gather on host
    ffw_eff = ff_w * na_g[None, :]          # fold LN(na) scale into ff
    ffb_eff = ff_b + ff_w @ na_b            # fold LN(na) bias into ff
    wv_eff = wv_w[0] * fn_g                 # fold LN(fn) scale into wv
    wvb_eff = wv_b[0] + wv_w[0] @ fn_b      # fold LN(fn) bias into wv
    # joint fold of wl_b and w5_b into the leaky shift bb and t4 bias B:
    #   bb = wl_b - wl_w @ (B - nf_b),  B - nf_b = w5_b + w5_w @ bb
    bb = np.linalg.solve(np.eye(3 * D) + wl_w @ w5_w, wl_b - wl_w @ w5_b)
    B_bias = nf_b + w5_b + w5_w @ bb

    wA = np.zeros((128, 256), np.float64)
    wA[:, A_ID:A_ID + 128] = np.eye(128)
    wA[0, A_FFB:A_FFB + 128] = ffb_eff

    wB = np.zeros((128, B_COLS), np.float64)
    wB[:, B_FFWT:B_FFWT + 128] = ffw_eff.T
    wB[:, B_WLWT:B_WLWT + 384] = wl_w.T
    wB[:, B_W5:B_W5 + 384] = w5_w.T.reshape(3, 128, 128).transpose(
        1, 0, 2).reshape(128, 384)
    wB[:, B_WVR:B_WVR + 128] = wv_eff[None, :]
    wB[:, B_NFG:B_NFG + 128] = nf_g[None, :]
    wB[:, B_NFB:B_NFB + 128] = B_bias[None, :]
    wB[:, B_BB3:B_BB3 + 3] = bb.reshape(3, 128).T
    wB[:, B_WVB] = wvb_eff

    f32 = lambda a: np.ascontiguousarray(a, dtype=np.float32)
    shared = {"wpacka": f32(wA), "wpackb": f32(wB)}
    in_maps = []
    for c in range(NCORES):
        xw = np.empty((128, 256), np.float64)
        xw[:, XW_XJT:XW_XJT + 128] = xj[c * PER:(c + 1) * PER].T
        xw[:, XW_W:XW_W + 128] = W
        m = dict(shared)
        m["xw"] = f32(xw)
        in_maps.append(m)
    return in_maps


def kernel(**inputs) -> np.ndarray:
    from concourse.bass_utils import run_bass_kernel_spmd

    nc = _get_nc()
    in_maps = _prep_in_maps(inputs)
    res = run_bass_kernel_spmd(nc, in_maps, core_ids=list(range(NCORES)))
    return np.concatenate(
        [np.asarray(res.results[c]["out"]).reshape(-1) for c in range(NCORES)]
    )


# revision 52
# speedup vs baseline: 1.2179x; 1.0591x over previous
"""Trainium2 Bass kernel for nn_AdjacencyGenerator (gnn_message_passing).

Math note (verified against the reference to ~5e-7 rel err):
  The reference builds att = softmax(..., axis=1) over an [E, E, D] tensor and
  then contracts it with einsum('ijk,il->ikl', att, Wh).  Since the j index
  appears only in att and softmax normalizes over j, sum_j att[i,j,k] == 1
  exactly, so h_prime[i,k,l] == Wh[i,l].  Every op after that point is
  row-wise over the [E*D, D] view, and row i*D+k of that view is Wh[i,:]
  independent of k.  The whole attention tensor therefore cancels and the
  output is a per-edge scalar o[i] = f(Wh[i,:]) repeated D times.

  f is: elu -> LN(na) -> ff linear -> leaky -> LN(nf) -> wl linear -> leaky
        -> w5 linear -> +residual -> LN(fn) -> wv linear.

  Exact algebraic folds used on the host (none are approximations):
    * na_g/na_b fold into ff_w/ff_b            (LN -> Linear)
    * fn_g/fn_b fold into wv_w/wv_b            (LN -> Linear)
    * wl_b and w5_b fold jointly into the leaky shift bb and the t4 bias B,
      solving (I + wl_w @ w5_w) bb = wl_b - wl_w @ w5_b on the host — this
      removes all wl/w5 bias matmuls exactly.
    * elu is computed as elu(x)+1 = exp(min(x,0)) + max(x,0); the +1 shift
      is constant along the normalized axis so the following LN cancels it.

  rstd(var) = exp(-0.5*ln(var+eps)) on the scalar engine: ln and exp live in
  the same ACT table set, so the whole kernel uses exactly one table load,
  pre-warmed off the critical path.

Distribution: shard the E=1024 edges 128 per core across 8 NeuronCores,
data-parallel; all weights replicated.  The edge gather x[edge_index[1]] is
part of input sharding, done on the host.  Inputs ship as three packed
images: [xjT|W] (per-core), [ident|ffb], and one [128, 1284] weight image.
"""

import numpy as np

D = 128
E = 1024
NCORES = 8
PER = E // NCORES  # 128 edges per core
EPS = 1e-5

# column offsets inside the packed images
XW_XJT, XW_W = 0, 128                      # d_xw [128, 256] (per-core)
A_ID, A_FFB = 0, 128                       # d_wA [128, 256]
B_FFWT, B_WLWT, B_W5, B_WVR, B_NFG, B_NFB, B_BB3, B_WVB = (
    0, 128, 512, 896, 1024, 1152, 1280, 1283)
B_COLS = 1284

_CACHE = {}


class _Seq:
    """Sequential instruction emitter for one engine with semaphore tags.

    attach=True (single-instruction ops, DVE/ACT): one wait rides on the
    instruction's own sync_info (HW allows a single attached wait); any
    extra waits are emitted standalone.  attach=False (multi-instruction
    groups like matmul, and DMA): all waits are standalone so they gate the
    whole group.
    """

    def __init__(self, eng, sem, all_self_waits, attach=False):
        self.eng, self.sem, self.n = eng, sem, 0
        self.all_self_waits = all_self_waits
        self.attach = attach

    def emit(self, make, waits=(), self_wait=False):
        allw = list(waits)
        if (self_wait or self.all_self_waits) and self.n:
            allw.append((self.sem, self.n))
        if self.attach and allw:
            for s, v in allw[:-1]:
                self.eng.wait_ge(s, v)
            inst = make()
            inst._wait_ge(*allw[-1])
        else:
            for s, v in allw:
                self.eng.wait_ge(s, v)
            inst = make()
        inst.then_inc(self.sem, 1)
        self.n += 1
        return self.n


def _build_nc(validation=False):
    import concourse.bass as bass
    from concourse import mybir

    f32 = mybir.dt.float32
    Alu = mybir.AluOpType
    Act = mybir.ActivationFunctionType

    nc = bass.Bass(detect_race_conditions=validation)

    d_xw = nc.dram_tensor("xw", [128, 256], f32, kind="ExternalInput")
    d_wA = nc.dram_tensor("wpacka", [128, 256], f32, kind="ExternalInput")
    d_wB = nc.dram_tensor("wpackb", [128, B_COLS], f32, kind="ExternalInput")
    d_out = nc.dram_tensor("out", [PER, D], f32, kind="ExternalOutput")

    from contextlib import ExitStack

    ctx = ExitStack()
    sb = lambda name, shape, dt=f32: ctx.enter_context(
        nc.sbuf_tensor(name, shape, dt))
    ps = lambda name, shape: ctx.enter_context(nc.psum_tensor(name, shape, f32))

    s_xj = sb("s_xj", [128, 128])
    s_w = sb("s_w", [128, 128])
    s_wA = sb("s_wa", [128, 256])
    s_wB = sb("s_wb", [128, B_COLS])

    ones = sb("ones", [1, 128])
    epsc = sb("epsc", [PER, 1])
    zeroc = sb("zeroc", [PER, 1])
    m0 = sb("m0", [PER, D])        # min(Wh, 0)
    ex = sb("ex", [PER, D])        # exp(min(Wh, 0))
    t1 = sb("t1", [PER, D])        # elu(Wh) + 1
    t2 = sb("t2", [PER, D])        # LN1 core
    t2T = sb("t2t", [D, PER])
    lk1 = sb("lk1", [PER, D])
    t3 = sb("t3", [PER, D])        # leaky(ff out)
    u = sb("u", [PER, D])          # LN2 core
    t4a = sb("t4a", [PER, D])
    t4 = sb("t4", [PER, D])
    t4T = sb("t4t", [D, PER])
    lka = sb("lka", [128, 3, PER])
    y1T = sb("y1t", [128, 3, PER])
    y3 = sb("y3", [PER, D])
    y4 = sb("y4", [PER, D])        # LN3 core
    y4w = sb("y4w", [PER, D])
    ocol = sb("ocol", [PER, 1])
    zerot = sb("zerot", [PER, D])
    o_sb = sb("o_sb", [PER, D])
    st = sb("st", [PER, 6])        # LN scratch (reused by all three LNs)
    mv = sb("mv", [PER, 2])
    lnv = sb("lnv", [PER, 1])
    rstd = sb("rstd", [PER, 1])
    scr = sb("scr", [1, 1])        # ACT warmup scratch

    p_wh = ps("p_wh", [PER, D])
    p_t2T = ps("p_t2t", [D, PER])
    p_q2 = ps("p_q2", [PER, D])
    p_t4T = ps("p_t4t", [D, PER])
    p_y1T = [ps(f"p_y1t{c}", [128, PER]) for c in range(3)]
    p_y2 = ps("p_y2", [PER, D])

    dsem_x = ctx.enter_context(nc.semaphore("dsem_x"))
    dsem_y = ctx.enter_context(nc.semaphore("dsem_y"))
    dsem_o = ctx.enter_context(nc.semaphore("dsem_o"))
    dsem_a = ctx.enter_context(nc.semaphore("dsem_a"))
    dsem_b = ctx.enter_context(nc.semaphore("dsem_b"))
    psem = ctx.enter_context(nc.semaphore("psem"))
    vsem = ctx.enter_context(nc.semaphore("vsem"))
    asem = ctx.enter_context(nc.semaphore("asem"))
    gsem = ctx.enter_context(nc.semaphore("gsem"))

    # ---- vector op indices ----------------------------------------------
    V_M0, V_T1 = 1, 2
    V_ST1, V_MV1, V_T2 = 3, 4, 5
    V_T2T, V_LK1, V_T3 = 6, 7, 8
    V_ST2, V_MV2, V_U = 9, 10, 11
    V_T4A, V_T4, V_T4T = 12, 13, 14
    V_Y1T = [16, 18, 20]
    V_Y3 = 21
    V_ST3, V_MV3, V_Y4 = 22, 23, 24
    V_Y4W, V_OCOL, V_OSB = 25, 26, 27
    # ---- PE op indices ---------------------------------------------------
    P_WH, P_Q2B, P_T2T, P_Q2, P_T4T = 1, 2, 3, 4, 5
    P_WL = [6, 7, 8]
    P_Y2 = [9, 10, 11]
    # ---- ACT op indices --------------------------------------------------
    A_WARM, A_EX = 1, 2
    A_R1, A_R2, A_R3 = 4, 6, 8
    # ---- gpsimd ----------------------------------------------------------
    G_ONES, G_SETUP = 1, 4

    with nc.Block() as block:

        @block.sync
        def _(sync):
            sync.dma_start(out=s_w[:, :], in_=d_xw[:, XW_W:XW_W + 128]
                           ).then_inc(dsem_y, 16)
            sync.dma_start(out=s_wA[:, :], in_=d_wA[:, :]).then_inc(dsem_a, 16)
            sync.dma_start(out=s_wB[:, :], in_=d_wB[:, :]).then_inc(dsem_b, 16)
            sync.wait_ge(vsem, V_OSB)
            sync.dma_start(out=d_out[:, :], in_=o_sb[:, :]).then_inc(dsem_o, 16)

        @block.gpsimd
        def _(ge):
            ge.dma_start(out=s_xj[:, :], in_=d_xw[:, XW_XJT:XW_XJT + 128]
                         ).then_inc(dsem_x, 16)
            ge.memset(ones[:, :], 1.0).then_inc(gsem, 1)
            ge.memset(epsc[:, :], EPS).then_inc(gsem, 1)
            ge.memset(zeroc[:, :], 0.0).then_inc(gsem, 1)
            ge.memset(zerot[:, :], 0.0).then_inc(gsem, 1)

        @block.scalar
        def _(se):
            A = _Seq(se, asem, validation, attach=True)
            # pre-warm the ln/exp table set off the critical path
            A.emit(lambda: se.activation(out=scr[:, :], in_=ones[0:1, 0:1],
                                         func=Act.Ln),
                   waits=[(gsem, G_ONES)])
            A.emit(lambda: se.activation(out=ex[:, :], in_=m0[:, :],
                                         func=Act.Exp),
                   waits=[(vsem, V_M0)])
            assert A.n == A_EX
            for a_idx, v_mv in ((A_R1, V_MV1), (A_R2, V_MV2), (A_R3, V_MV3)):
                # rstd = exp(-0.5 * ln(var + eps))
                A.emit(lambda v_mv=v_mv: se.activation(
                    out=lnv[:, :], in_=mv[:, 1:2], func=Act.Ln,
                    bias=epsc[:, 0:1]),
                    waits=[(vsem, v_mv)])
                A.emit(lambda: se.activation(out=rstd[:, :], in_=lnv[:, :],
                                             func=Act.Exp, scale=-0.5),
                       self_wait=True)
                assert A.n == a_idx

        @block.tensor
        def _(te):
            T = _Seq(te, psem, validation)
            # Wh = xj @ W  (xjT and W arrive on different DMA rings)
            T.emit(lambda: te.matmul(p_wh[:, :], s_xj[:, :], s_w[:, :],
                                     start=True, stop=True),
                   waits=[(dsem_x, 16), (dsem_y, 16)])
            # ff bias early (its only deps are DMA + ones memset)
            T.emit(lambda: te.matmul(p_q2[:, :], ones[:, :],
                                     s_wA[0:1, A_FFB:A_FFB + 128],
                                     start=True, stop=False,
                                     skip_group_check=True),
                   waits=[(dsem_a, 16), (dsem_b, 16), (gsem, G_ONES)])
            T.emit(lambda: te.transpose(p_t2T[:, :], t2[:, :],
                                        s_wA[:, A_ID:A_ID + 128]),
                   waits=[(vsem, V_T2)])
            T.emit(lambda: te.matmul(p_q2[:, :], t2T[:, :],
                                     s_wB[:, B_FFWT:B_FFWT + 128],
                                     start=False, stop=True,
                                     skip_group_check=True),
                   waits=[(vsem, V_T2T)])
            T.emit(lambda: te.transpose(p_t4T[:, :], t4[:, :],
                                        s_wA[:, A_ID:A_ID + 128]),
                   waits=[(vsem, V_T4)])
            for c in range(3):
                T.emit(lambda c=c: te.matmul(
                    p_y1T[c][:, :],
                    s_wB[:, B_WLWT + c * 128:B_WLWT + (c + 1) * 128],
                    t4T[:, :], start=True, stop=True),
                    waits=[(vsem, V_T4T)] if c == 0 else ())
            for c in range(3):
                T.emit(lambda c=c: te.matmul(
                    p_y2[:, :], y1T[:, c, :],
                    s_wB[:, B_W5 + c * 128:B_W5 + (c + 1) * 128],
                    start=(c == 0), stop=(c == 2)),
                    waits=[(vsem, V_Y1T[c])])
            assert T.n == P_Y2[2]

        @block.vector
        def _(ve):
            V = _Seq(ve, vsem, validation, attach=True)
            V.emit(lambda: ve.tensor_scalar_min(out=m0[:, :], in0=p_wh[:, :],
                                                scalar1=0.0),
                   waits=[(psem, P_WH), (gsem, G_SETUP)])
            V.emit(lambda: ve.scalar_tensor_tensor(out=t1[:, :], in0=p_wh[:, :],
                                                   scalar=0.0, in1=ex[:, :],
                                                   op0=Alu.max, op1=Alu.add),
                   waits=[(asem, A_EX)])
            assert V.n == V_T1

            def ln_core(src, dst, a_idx, v_stats):
                V.emit(lambda: ve.bn_stats(out=st[:, :], in_=src[:, :]))
                V.emit(lambda: ve.bn_aggr(out=mv[:, :], in_=st[:, :]),
                       self_wait=True)
                assert V.n == v_stats + 1
                # scalar operands latch at dispatch; the asem wait (ACT wrote
                # rstd) transitively guarantees mv is long since drained
                V.emit(lambda: ve.tensor_scalar(out=dst[:, :], in0=src[:, :],
                                                scalar1=mv[:, 0:1],
                                                scalar2=rstd[:, 0:1],
                                                op0=Alu.subtract,
                                                op1=Alu.mult),
                       waits=[(asem, a_idx)])

            ln_core(t1, t2, A_R1, V_ST1)
            assert V.n == V_T2
            V.emit(lambda: ve.tensor_copy(out=t2T[:, :], in_=p_t2T[:, :]),
                   waits=[(psem, P_T2T)])
            # leaky(q2) = q2 - 0.8*min(q2, 0)
            V.emit(lambda: ve.tensor_scalar(out=lk1[:, :], in0=p_q2[:, :],
                                            scalar1=0.0, scalar2=0.8,
                                            op0=Alu.min, op1=Alu.mult),
                   waits=[(psem, P_Q2)])
            V.emit(lambda: ve.tensor_sub(out=t3[:, :], in0=p_q2[:, :],
                                         in1=lk1[:, :]))
            assert V.n == V_T3
            ln_core(t3, u, A_R2, V_ST2)
            assert V.n == V_U
            # t4 = u * nf_g + B
            V.emit(lambda: ve.tensor_mul(out=t4a[:, :], in0=u[:, :],
                                         in1=s_wB[:, B_NFG:B_NFG + 128]))
            V.emit(lambda: ve.tensor_add(out=t4[:, :], in0=t4a[:, :],
                                         in1=s_wB[:, B_NFB:B_NFB + 128]))
            V.emit(lambda: ve.tensor_copy(out=t4T[:, :], in_=p_t4T[:, :]),
                   waits=[(psem, P_T4T)])
            assert V.n == V_T4T
            # leaky with folded bias, per chunk (each wl output has its
            # own PSUM bank, so chunk c can be read while the PE writes c+1):
            #   y1T_c = mm_c - 0.8*min(mm_c + bb_c, 0)
            for c in range(3):
                bb_c = s_wB[:, B_BB3 + c:B_BB3 + c + 1]
                V.emit(lambda c=c, bb_c=bb_c: ve.tensor_scalar(
                    out=lka[:, c, :], in0=p_y1T[c][:, :],
                    scalar1=bb_c, scalar2=zeroc[:, 0:1],
                    op0=Alu.add, op1=Alu.min),
                    waits=[(psem, P_WL[c])])
                V.emit(lambda c=c: ve.scalar_tensor_tensor(
                    out=y1T[:, c, :], in0=lka[:, c, :], scalar=-0.8,
                    in1=p_y1T[c][:, :], op0=Alu.mult, op1=Alu.add))
                assert V.n == V_Y1T[c]
            V.emit(lambda: ve.tensor_add(out=y3[:, :], in0=p_y2[:, :],
                                         in1=t4[:, :]),
                   waits=[(psem, P_Y2[2])])
            assert V.n == V_Y3
            # LN3 fused with the wv dot product:
            #   out[e] = rstd3[e] * sum_k ((y3-m3)[e,k]*wv_eff[k]) + wvb
            # the (y3-m)*wv part runs on DVE while ACT computes rstd3
            V.emit(lambda: ve.bn_stats(out=st[:, :], in_=y3[:, :]))
            V.emit(lambda: ve.bn_aggr(out=mv[:, :], in_=st[:, :]),
                   self_wait=True)
            assert V.n == V_MV3
            V.emit(lambda: ve.scalar_tensor_tensor(
                out=y4w[:, :], in0=y3[:, :], scalar=mv[:, 0:1],
                in1=s_wB[:, B_WVR:B_WVR + 128],
                op0=Alu.subtract, op1=Alu.mult),
                self_wait=True)
            V.emit(lambda: ve.tensor_reduce(out=ocol[:, :], in_=y4w[:, :],
                                            axis=mybir.AxisListType.X,
                                            op=Alu.add))
            V.emit(lambda: ve.tensor_scalar(out=ocol[:, :], in0=ocol[:, :],
                                            scalar1=rstd[:, 0:1],
                                            scalar2=s_wB[:, B_WVB:B_WVB + 1],
                                            op0=Alu.mult, op1=Alu.add),
                   waits=[(asem, A_R3)])
            V.emit(lambda: ve.tensor_scalar_add(out=o_sb[:, :],
                                                in0=zerot[:, :],
                                                scalar1=ocol[:, 0:1]),
                   self_wait=True)
            assert V.n == V_OSB

    return nc, ctx


def _get_nc(validation=False):
    key = "ncv" if validation else "nc"
    if key not in _CACHE:
        _CACHE[key] = _build_nc(validation)
    return _CACHE[key][0]


def _prep_in_maps(inputs):
    """Host-side sharding + exact algebraic weight folding + packing."""
    g = lambda k: np.asarray(inputs[k], dtype=np.float64)
    x = g("x")
    ei = np.asarray(inputs["edge_index"]).astype(np.int64)
    W = g("W")
    ff_w, ff_b = g("ff_w"), g("ff_b")
    na_g, na_b = g("na_g"), g("na_b")
    nf_g, nf_b = g("nf_g"), g("nf_b")
    wl_w, wl_b = g("wl_w"), g("wl_b")
    w5_w, w5_b = g("w5_w"), g("w5_b")
    fn_g, fn_b = g("fn_g"), g("fn_b")
    wv_w, wv_b = g("wv_w"), g("wv_b")

    xj = x[ei[1]]                           # [E, D] gather on host
    ffw_eff = ff_w * na_g[None, :]          # fold LN(na) scale into ff
    ffb_eff = ff_b + ff_w @ na_b            # fold LN(na) bias into ff
    wv_eff = wv_w[0] * fn_g                 # fold LN(fn) scale into wv
    wvb_eff = wv_b[0] + wv_w[0] @ fn_b      # fold LN(fn) bias into wv
    # joint fold of wl_b and w5_b into the leaky shift bb and t4 bias B:
    #   bb = wl_b - wl_w @ (B - nf_b),  B - nf_b = w5_b + w5_w @ bb
    bb = np.linalg.solve(np.eye(3 * D) + wl_w @ w5_w, wl_b - wl_w @ w5_b)
    B_bias = nf_b + w5_b + w5_w @ bb

    wA = np.zeros((128, 256), np.float64)
    wA[:, A_ID:A_ID + 128] = np.eye(128)
    wA[0, A_FFB:A_FFB + 128] = ffb_eff

    wB = np.zeros((128, B_COLS), np.float64)
    wB[:, B_FFWT:B_FFWT + 128] = ffw_eff.T
    wB[:, B_WLWT:B_WLWT + 384] = wl_w.T
    wB[:, B_W5:B_W5 + 384] = w5_w.T.reshape(3, 128, 128).transpose(
        1, 0, 2).reshape(128, 384)
    wB[:, B_WVR:B_WVR + 128] = wv_eff[None, :]
    wB[:, B_NFG:B_NFG + 128] = nf_g[None, :]
    wB[:, B_NFB:B_NFB + 128] = B_bias[None, :]
    wB[:, B_BB3:B_BB3 + 3] = bb.reshape(3, 128).T
    wB[:, B_WVB] = wvb_eff

    f32 = lambda a: np.ascontiguousarray(a, dtype=np.float32)
    shared = {"wpacka": f32(wA), "wpackb": f32(wB)}
    in_maps = []
    for c in range(NCORES):
        xw = np.empty((128, 256), np.float64)
        xw[:, XW_XJT:XW_XJT + 128] = xj[c * PER:(c + 1) * PER].T
        xw[:, XW_W:XW_W + 128] = W
        m = dict(shared)
        m["xw"] = f32(xw)
        in_maps.append(m)
    return in_maps


def kernel(**inputs) -> np.ndarray:
    from concourse.bass_utils import run_bass_kernel_spmd

    nc = _get_nc()
    in_maps = _prep_in_maps(inputs)
    res = run_bass_kernel_spmd(nc, in_maps, core_ids=list(range(NCORES)))
    return np.concatenate(
        [np.asarray(res.results[c]["out"]).reshape(-1) for c in range(NCORES)]
    )
